# revision 7
# baseline (speedup 1.0000x reference)
"""Trainium2 Bass kernel for MultiHeadEdgeAwareMessagePassing.

Math restructure (exact up to a ~1e-6 Taylor truncation):
  logits[i,j,h] = s_q[i,h] + s_k[j,h] + w[i,j]*c1[h] + c0[h]   (valid j: w>0)
  alpha = softmax_j(logits) * w
The s_q and c0 terms are constant over j and cancel in the softmax, so
  msg[i,h,:] = Num_h[i,:] / Den_h[i]
  Num_h[i,d] = sum_j [w>0] * w * exp(c1_h w) * g_h[j] * v_h[j,d]
  Den_h[i]   = sum_j [w>0] *     exp(c1_h w) * g_h[j]
with g[j,h] = exp(s_k[j,h]).  |c1_h * w| <~ 0.02, so
  exp(c1 w) ~= 1 + c1 w + c1^2 w^2 / 2
Defining head-independent matrices  mask=[w>0], W1=relu(w), W2=relu(w)*w:
  Num_h = (sum_j W1^T (g_h v_h)) + c1_h (sum_j W2^T (g_h v_h))
  Den_h = (sum_j mask^T g_h) + c1_h (sum_j W1^T g_h) + c1_h^2/2 (sum_j W2^T g_h)
These are plain matmuls with a shared 260-column rhs = [g*v (256) | g (4)].

Sharding: destination rows i split across 8 cores (384 rows each). Each core
reads its [3072, 384] slice of w^T, plus replicated h (transposed) and the
small projection weights. Transposition of w/h is host-side layout prep; all
compute (including projections, output proj, layernorm) runs on device.
"""

import numpy as np

N = 3072
D = 256
H = 4
DH = 64
DE = 8
NCORES = 8
ISLICE = N // NCORES  # 384
NJT = N // 128  # 24
NSUB = ISLICE // 128  # 3

_cache = {}


def _build_bass():
    import concourse.bass as bass
    import concourse.tile as tile
    from concourse import bacc, mybir
    from concourse.bass import ts
    from concourse.masks import make_identity

    dt = mybir.dt
    AF = mybir.ActivationFunctionType
    OP = mybir.AluOpType

    nc = bacc.Bacc("TRN2", target_bir_lowering=False, debug=False,
                   num_devices=NCORES)

    # ---- DRAM I/O (per-core views; host does the slicing) ----
    wt_d = nc.dram_tensor("wt", [N, ISLICE], dt.float32, kind="ExternalInput")
    ht_d = nc.dram_tensor("ht", [D, N], dt.float32, kind="ExternalInput")
    hs_d = nc.dram_tensor("hs", [ISLICE, D], dt.float32, kind="ExternalInput")
    Wk_d = nc.dram_tensor("Wk", [D, D], dt.float32, kind="ExternalInput")
    WvT_d = nc.dram_tensor("WvT", [D, D], dt.float32, kind="ExternalInput")
    WoT_d = nc.dram_tensor("WoT", [D, D], dt.float32, kind="ExternalInput")
    u_d = nc.dram_tensor("u", [H, 2 * DH + DE], dt.float32, kind="ExternalInput")
    bk_d = nc.dram_tensor("bk", [D], dt.float32, kind="ExternalInput")
    bv_d = nc.dram_tensor("bv", [D], dt.float32, kind="ExternalInput")
    bo_d = nc.dram_tensor("bo", [D], dt.float32, kind="ExternalInput")
    Wew_d = nc.dram_tensor("Wew", [H * DE, 1], dt.float32, kind="ExternalInput")
    gam_d = nc.dram_tensor("gamma", [D], dt.float32, kind="ExternalInput")
    bet_d = nc.dram_tensor("beta", [D], dt.float32, kind="ExternalInput")
    out_d = nc.dram_tensor("out", [ISLICE, D], dt.float32, kind="ExternalOutput")

    bf = dt.bfloat16
    f32 = dt.float32

    with tile.TileContext(nc) as tc:
        with (
            tc.tile_pool(name="consts", bufs=1) as consts,
            tc.tile_pool(name="wtp", bufs=3) as wtp,
            tc.tile_pool(name="elem", bufs=2) as elem,
            tc.tile_pool(name="rhsp", bufs=3) as rhsp,
            tc.tile_pool(name="gp", bufs=2) as gp,
            tc.tile_pool(name="small", bufs=4) as small,
            tc.tile_pool(name="outp", bufs=2) as outp,
            tc.tile_pool(name="acc", bufs=1, space="PSUM") as accp,
            tc.tile_pool(name="pre", bufs=2, space="PSUM") as prep,
        ):
            # ---------------- constants / setup ----------------
            ht_sb = consts.tile([128, 2, N], bf, tag="ht")
            for a in range(2):
                nc.gpsimd.dma_start(ht_sb[:, a, :], ht_d[128 * a:128 * (a + 1), :])

            Wk_sb = consts.tile([128, 2, D], bf, tag="wk")
            for a in range(2):
                nc.gpsimd.dma_start(Wk_sb[:, a, :], Wk_d[128 * a:128 * (a + 1), :])

            rhs_vk = consts.tile([128, 2, 260], bf, tag="rhsvk")
            for a in range(2):
                nc.gpsimd.dma_start(rhs_vk[:, a, 0:256],
                                    WvT_d[128 * a:128 * (a + 1), :])

            WoT_sb = consts.tile([128, 2, D], bf, tag="wot")
            for a in range(2):
                nc.gpsimd.dma_start(WoT_sb[:, a, :], WoT_d[128 * a:128 * (a + 1), :])

            # U[r, h] = u_k[h, d] at r = h*64+d  (for a_k^T = Wk^T-weighted u_k)
            U_sb = consts.tile([128, 2, H], bf, tag="U")
            nc.vector.memset(U_sb, 0.0)
            for h in range(H):
                p0 = (h * DH) % 128
                a = (h * DH) // 128
                nc.gpsimd.dma_start(
                    U_sb[p0:p0 + DH, a, h:h + 1],
                    u_d[h, DH:2 * DH].rearrange("(f o) -> f o", o=1))

            # Ue[r, h] = u_e[h, d] at r = h*8+d (for c1)
            Ue_sb = consts.tile([H * DE, H], bf, tag="Ue")
            nc.vector.memset(Ue_sb, 0.0)
            for h in range(H):
                nc.gpsimd.dma_start(
                    Ue_sb[h * DE:(h + 1) * DE, h:h + 1],
                    u_d[h, 2 * DH:2 * DH + DE].rearrange("(f o) -> f o", o=1))

            Wew_sb = consts.tile([H * DE, 1], bf, tag="wew")
            nc.gpsimd.dma_start(Wew_sb, Wew_d.ap())

            bk_sb = consts.tile([128, 2], bf, tag="bk")
            nc.gpsimd.dma_start(bk_sb, bk_d.ap().rearrange("(a p) -> p a", p=128))

            ones_sb = consts.tile([1, 128], bf, tag="ones")
            nc.vector.memset(ones_sb, 1.0)

            rhs_bias = consts.tile([1, 260], bf, tag="rhsbias")
            nc.gpsimd.dma_start(rhs_bias[0:1, 0:256],
                                bv_d.ap().rearrange("(o f) -> o f", o=1))

            bo_row = consts.tile([1, 256], bf, tag="borow")
            nc.gpsimd.dma_start(bo_row, bo_d.ap().rearrange("(o f) -> o f", o=1))

            ident = consts.tile([128, 128], bf, tag="ident")
            make_identity(nc, ident)

            gam_sb = consts.tile([128, D], f32, tag="gam")
            nc.gpsimd.dma_start(
                gam_sb, bass.AP(tensor=gam_d, offset=0, ap=[[0, 128], [1, D]]))
            bet_sb = consts.tile([128, D], f32, tag="bet")
            nc.gpsimd.dma_start(
                bet_sb, bass.AP(tensor=bet_d, offset=0, ap=[[0, 128], [1, D]]))

            eps_sb = consts.tile([128, 1], f32, tag="eps")
            nc.vector.memset(eps_sb, 1e-5)

            # b_k[h] = sum_d u_k[h,d] bk[h*64+d]
            ps_bk = prep.tile([1, H], f32, tag="pre")
            for a in range(2):
                nc.tensor.matmul(ps_bk, bk_sb[:, a:a + 1], U_sb[:, a, :],
                                 start=(a == 0), stop=(a == 1))
            nc.vector.tensor_copy(rhs_bias[0:1, 256:260], ps_bk)

            # a_k^T[dm, h] = sum_r Wk[r, dm] U[r, h]
            for b in range(2):
                ps_ak = prep.tile([128, H], f32, tag="pre")
                for a in range(2):
                    nc.tensor.matmul(ps_ak, Wk_sb[:, a, 128 * b:128 * (b + 1)],
                                     U_sb[:, a, :], start=(a == 0), stop=(a == 1))
                nc.vector.tensor_copy(rhs_vk[:, b, 256:260], ps_ak)

            # c1[h] = sum_r We_w[r,0] Ue[r,h]; broadcast to 128 partitions
            ps_c1 = prep.tile([1, H], f32, tag="pre")
            nc.tensor.matmul(ps_c1, Wew_sb, Ue_sb, start=True, stop=True)
            c1row = consts.tile([1, H], bf, tag="c1row")
            nc.vector.tensor_copy(c1row, ps_c1)
            ps_c1b = prep.tile([128, H], f32, tag="pre")
            nc.tensor.matmul(ps_c1b, ones_sb, c1row, start=True, stop=True)
            c1b = consts.tile([128, H], f32, tag="c1b")
            nc.vector.tensor_copy(c1b, ps_c1b)
            c1sqh = consts.tile([128, H], f32, tag="c1sqh")
            nc.vector.tensor_mul(c1sqh, c1b, c1b)
            nc.vector.tensor_scalar_mul(c1sqh, c1sqh, 0.5)

            # ---------------- persistent accumulators ----------------
            psA = [accp.tile([128, 264], f32, tag=f"A{s}", name=f"psA{s}")
                   for s in range(NSUB)]
            psB = [accp.tile([128, 260], f32, tag=f"B{s}", name=f"psB{s}")
                   for s in range(NSUB)]

            # ---------------- main loop over source tiles ----------------
            for jt in range(NJT):
                # v and s_k for this j-tile:  [128j, 260] = [v | s_k]
                ps_pre = prep.tile([128, 260], f32, tag="pre")
                for a in range(2):
                    nc.tensor.matmul(ps_pre, ht_sb[:, a, ts(jt, 128)],
                                     rhs_vk[:, a, :], start=(a == 0), stop=False)
                nc.tensor.matmul(ps_pre, ones_sb, rhs_bias,
                                 start=False, stop=True)

                g32 = gp.tile([128, H], f32, tag="g32")
                nc.scalar.activation(g32, ps_pre[:, 256:260], AF.Exp)

                rhs_big = rhsp.tile([128, 260], bf, tag="rhsbig")
                for h in range(H):
                    nc.vector.tensor_scalar(
                        rhs_big[:, h * DH:(h + 1) * DH],
                        ps_pre[:, h * DH:(h + 1) * DH],
                        g32[:, h:h + 1], None, op0=OP.mult)
                nc.vector.tensor_copy(rhs_big[:, 256:260], g32)

                # w^T tile and its elementwise powers
                wt_t = wtp.tile([128, ISLICE], f32, tag="wt")
                nc.sync.dma_start(wt_t, wt_d[ts(jt, 128), :])

                W1 = elem.tile([128, ISLICE], bf, tag="W1")
                nc.scalar.activation(W1, wt_t, AF.Relu)
                msk = elem.tile([128, ISLICE], bf, tag="msk")
                nc.vector.tensor_scalar(msk, wt_t, 0.0, None, op0=OP.is_gt)
                W2 = elem.tile([128, ISLICE], bf, tag="W2")
                nc.scalar.square(W2, W1)

                st = (jt == 0)
                sp = (jt == NJT - 1)
                for s in range(NSUB):
                    sl = ts(s, 128)
                    nc.tensor.matmul(psA[s][:, 0:260], W1[:, sl], rhs_big,
                                     start=st, stop=sp)
                    nc.tensor.matmul(psB[s][:, 0:260], W2[:, sl], rhs_big,
                                     start=st, stop=sp)
                    nc.tensor.matmul(psA[s][:, 260:264], msk[:, sl],
                                     rhs_big[:, 256:260], start=st, stop=sp)

            # ---------------- epilogue per destination subtile ----------------
            for s in range(NSUB):
                # Den_h = mask.g + c1*W1.g + (c1^2/2)*W2.g
                den = small.tile([128, H], f32, tag="den")
                nc.vector.tensor_mul(den, c1b, psA[s][:, 256:260])
                nc.vector.tensor_add(den, den, psA[s][:, 260:264])
                t2 = small.tile([128, H], f32, tag="t2")
                nc.vector.tensor_mul(t2, c1sqh, psB[s][:, 256:260])
                nc.vector.tensor_add(den, den, t2)
                rden = small.tile([128, H], f32, tag="rden")
                nc.vector.reciprocal(rden, den)
                rdc1 = small.tile([128, H], f32, tag="rdc1")
                nc.vector.tensor_mul(rdc1, rden, c1b)

                # msg_h = A_h*rden_h + B_h*(c1_h*rden_h)   -> bf16
                msgA = outp.tile([128, D], bf, tag="msgA")
                msgB = outp.tile([128, D], bf, tag="msgB")
                msg = outp.tile([128, D], bf, tag="msg")
                for h in range(H):
                    hsl = slice(h * DH, (h + 1) * DH)
                    nc.vector.tensor_scalar(msgA[:, hsl], psA[s][:, hsl],
                                            rden[:, h:h + 1], None, op0=OP.mult)
                    nc.vector.tensor_scalar(msgB[:, hsl], psB[s][:, hsl],
                                            rdc1[:, h:h + 1], None, op0=OP.mult)
                nc.vector.tensor_add(msg, msgA, msgB)

                # transpose msg -> [dm, i] for the output projection
                msgT = outp.tile([128, 2, 128], bf, tag="msgT")
                for b in range(2):
                    ps_t = prep.tile([128, 128], bf, tag="pre")
                    nc.tensor.transpose(ps_t, msg[:, ts(b, 128)], ident)
                    nc.vector.tensor_copy(msgT[:, b, :], ps_t)

                # out = msg @ Wo^T + bo
                ps_o = prep.tile([128, D], f32, tag="pre")
                nc.tensor.matmul(ps_o, msgT[:, 0, :], WoT_sb[:, 0, :],
                                 start=True, stop=False)
                nc.tensor.matmul(ps_o, msgT[:, 1, :], WoT_sb[:, 1, :],
                                 start=False, stop=False)
                nc.tensor.matmul(ps_o, ones_sb, bo_row, start=False, stop=True)

                # x = h + out; layernorm
                x = outp.tile([128, D], f32, tag="x")
                hseg = outp.tile([128, D], f32, tag="hseg")
                nc.sync.dma_start(hseg, hs_d[ts(s, 128), :])
                nc.vector.tensor_add(x, ps_o, hseg)

                stats = small.tile([128, 6], f32, tag="stats")
                nc.vector.bn_stats(out=stats, in_=x)
                mv = small.tile([128, 2], f32, tag="mv")
                nc.vector.bn_aggr(out=mv, in_=stats)
                sd = small.tile([128, 1], f32, tag="sd")
                nc.scalar.activation(sd, mv[:, 1:2], AF.Sqrt, bias=eps_sb)
                rstd = small.tile([128, 1], f32, tag="rstd")
                nc.vector.reciprocal(rstd, sd)

                y = outp.tile([128, D], f32, tag="y")
                nc.vector.tensor_scalar(y, x, mv[:, 0:1], rstd,
                                        op0=OP.subtract, op1=OP.mult)
                ot = outp.tile([128, D], f32, tag="ot")
                nc.vector.tensor_mul(ot, y, gam_sb)
                nc.vector.tensor_add(ot, ot, bet_sb)
                nc.sync.dma_start(out_d[ts(s, 128), :], ot)

    nc.compile()
    return nc


def kernel(h, w, Wq, bq, Wk, bk, Wv, bv, We_w, We_b, u, Wo, bo, gamma, beta,
           **_unused):
    from concourse.bass_utils import run_bass_kernel_spmd

    if "nc" not in _cache:
        _cache["nc"] = _build_bass()
    nc = _cache["nc"]

    f = np.float32
    h = np.ascontiguousarray(h, dtype=f)
    wT = np.ascontiguousarray(np.asarray(w, dtype=f).T)
    hT = np.ascontiguousarray(h.T)
    WvT = np.ascontiguousarray(np.asarray(Wv, dtype=f).T)
    WoT = np.ascontiguousarray(np.asarray(Wo, dtype=f).T)
    common = {
        "ht": hT,
        "Wk": np.ascontiguousarray(Wk, dtype=f),
        "WvT": WvT,
        "WoT": WoT,
        "u": np.ascontiguousarray(u, dtype=f),
        "bk": np.ascontiguousarray(bk, dtype=f),
        "bv": np.ascontiguousarray(bv, dtype=f),
        "bo": np.ascontiguousarray(bo, dtype=f),
        "Wew": np.ascontiguousarray(We_w, dtype=f),
        "gamma": np.ascontiguousarray(gamma, dtype=f),
        "beta": np.ascontiguousarray(beta, dtype=f),
    }
    in_maps = []
    for c in range(NCORES):
        sl = slice(c * ISLICE, (c + 1) * ISLICE)
        m = dict(common)
        m["wt"] = np.ascontiguousarray(wT[:, sl])
        m["hs"] = np.ascontiguousarray(h[sl, :])
        in_maps.append(m)

    res = run_bass_kernel_spmd(nc, in_maps, core_ids=list(range(NCORES)))
    out = np.concatenate([r["out"] for r in res.results], axis=0)
    return np.ascontiguousarray(out, dtype=np.float32)
